# revision 31
# baseline (speedup 1.0000x reference)
"""Multi-head causal attention (B=2, S=2048, D=1024, H=16) on 8 NeuronCores.

Sharding v3: 2-way data parallel over batch x 4-way tensor parallel over
heads (core c handles batch c//4, heads 4*(c%4)..4*(c%4)+3). Each core
computes q/k/v projections for its 4 heads over its batch's 2048 tokens,
causal attention, and a partial output projection (its 256 rows of
W_proj); the host sums 4 partials per batch and adds b_proj.

Device-side design (all matmuls bf16 with fp32 PSUM accumulate):
 - x arrives pre-transposed and tiled [128, 4 col-tiles, 8 ks, 512] so
   every input DMA chunk is >=2KB-contiguous per partition.
 - q is produced transposed in two 2-head tiles ([128, 2048] each); k
   likewise but stored as FOUR zero-padded copies (head h in rows
   64*(h%2).., zeros elsewhere) so each head's score matmul contracts
   K=128 at the full-array rate.
 - scores are computed as ST = K @ Q^T ([keys, queries], 512-wide query
   groups); one Exp instruction per kj-pair psum tile. exp runs on
   ScalarE; q/k/v copyback on DVE; causal-triangle masking on the
   otherwise-idle GpSimd, emitted per-pair so AV never waits on it.
 - v is produced token-major [tokens, 260] = [V_h|1]x4; the AV product
   expST.T @ [V|1] yields context AND the softmax denominator in one
   accumulation group with queries on PSUM partitions. The 4 query
   tiles of a group accumulate into ONE psum bank ([128,4,66]) so a
   single reciprocal serves all 4 normalizations.
 - scores/AV are software-pipelined ACROSS HEADS; lagged output groups
   are emitted between AV(1,g) and S(3,g) so their DVE casts queue
   after the norms that feed the next transposes (avoids a DVE-order
   priority inversion that stalls the output projection).
 - output is stored partition-major ([128, 16*1024] per core) so out
   DMAs have 2-8KB contiguous descriptors; the host undoes the tiling.
   The last group runs a fused per-tile drain: AV chain -> norm ->
   transpose -> out-proj -> half-tile DMA, pipelined per token tile.
"""

import sys

sys.path.insert(0, "/opt/trn_rl_repo")

import numpy as np
import ml_dtypes

import concourse.bass as bass
import concourse.mybir as mybir
import concourse.tile as tile
from concourse import bacc
from concourse.bass_utils import run_bass_kernel_spmd

BF16 = mybir.dt.bfloat16
F32 = mybir.dt.float32
NPBF16 = ml_dtypes.bfloat16

B, S, D = 2, 2048, 1024
H, DH = 16, 64
HC = 4               # heads per core
T = S                # tokens per core (one batch)
KS = D // 128        # 8 contraction subtiles
QT = T // 128        # 16 query tiles
NCOL = 4             # 512-token projection column tiles
ACT_F = mybir.ActivationFunctionType


def _build_nc():
    # Bacc (not raw Bass): its compile() pass pipeline splits multi-sem
    # waits down to the TRN2 1-wait-per-instruction hardware limit.
    nc = bacc.Bacc("TRN2", target_bir_lowering=False, debug=False, num_devices=8)

    xT = nc.dram_tensor("xT", [128, NCOL, KS, 512], BF16, kind="ExternalInput")
    wq = nc.dram_tensor("wq", [128, 2, KS, 128], BF16, kind="ExternalInput")
    wk = nc.dram_tensor("wk", [128, 2, KS, 128], BF16, kind="ExternalInput")
    wv = nc.dram_tensor("wv", [128, KS, 260], BF16, kind="ExternalInput")
    bq = nc.dram_tensor("bq", [128, 2], F32, kind="ExternalInput")
    bk = nc.dram_tensor("bk", [128, 2], F32, kind="ExternalInput")
    bv = nc.dram_tensor("bv", [1, 260], BF16, kind="ExternalInput")
    wp = nc.dram_tensor("wp", [128, 2, D], BF16, kind="ExternalInput")
    tri = nc.dram_tensor("tri", [128, 128], BF16, kind="ExternalInput")
    # partition-major output: o[p, tt*1024 + d] = out[tt*128 + p, d]
    out = nc.dram_tensor("o", [128, QT * D], BF16, kind="ExternalOutput")

    with tile.TileContext(nc) as tc:
        with (
            tc.tile_pool(name="singles", bufs=1) as singles,
            # one psum pool: tag "qk" [128,2,512] f32 = 2 banks x 2 bufs,
            # tag "av" [128,4,66] = 1 bank x 2, tag "po" [128,512] = 1 bank
            # x 2 -> exactly 8 banks
            tc.tile_pool(name="qkps", bufs=2, space="PSUM") as qkps,
            tc.tile_pool(name="expp", bufs=36) as expp,
            tc.tile_pool(name="ctxp", bufs=6) as ctxp,
            tc.tile_pool(name="outp", bufs=2) as outp,
            tc.tile_pool(name="rdp", bufs=4) as rdp,
        ):
            # ---- resident tensors -------------------------------------
            wq_sb = singles.tile([128, 2, KS, 128], BF16, tag="wq")
            wk_sb = singles.tile([128, 2, KS, 128], BF16, tag="wk")
            wv_sb = singles.tile([128, KS, 260], BF16, tag="wv")
            bq_sb = singles.tile([128, 2], F32, tag="bq")
            bk_sb = singles.tile([128, 2], F32, tag="bk")
            # b_v (+ the ones columns) broadcast to all partitions; fused
            # into the v copyback as a tensor_tensor add on DVE
            bv_sb = singles.tile([128, 260], BF16, tag="bv")
            wp_sb = singles.tile([128, 2, D], BF16, tag="wp")
            tri_sb = singles.tile([128, 128], BF16, tag="tri")
            xT_sb = singles.tile([128, NCOL, KS, 512], BF16, tag="xT")
            # q for heads (0,1) in qT[0] rows (0:64|64:128), (2,3) in qT[1]
            qT = [
                singles.tile([128, T], BF16, tag=f"qT{i}", name=f"qT{i}")
                for i in range(2)
            ]
            # kT stored 4x, zero-padded per head (see module docstring)
            kTz = [
                singles.tile([128, T], BF16, tag=f"kTz{h}", name=f"kTz{h}")
                for h in range(HC)
            ]
            # v, per key-tile: [V_h0 | 1 | V_h1 | 1 | V_h2 | 1 | V_h3 | 1]
            v_sb = singles.tile([128, QT, 260], BF16, tag="v")
            # ctxT: dims of heads (0,1) in [0], (2,3) in [1]; matches wp rows
            ctxT = [
                singles.tile([128, QT, 128], BF16, tag=f"ctxT{i}", name=f"ctxT{i}")
                for i in range(2)
            ]

            # ---- input DMA: sync queue carries everything the first
            # q-projection chain needs, in need-order (wq then xT col 0
            # then later cols); scalar queue carries the small consts
            # then the big weights needed a bit later (wk by ~15us)
            # three queues pull the critical first-column data in
            # parallel: sync carries wqA/wkA + the front of xT col 0,
            # gpsimd (25ns issue, before its memsets) the back half of
            # col 0, scalar the consts then half-1 weights then wv
            nc.gpsimd.dma_start(xT_sb[:, 0, 5:, :], xT[:, 0, 5:, :])
            nc.sync.dma_start(wq_sb[:, 0, :, :], wq[:, 0, :, :])
            nc.sync.dma_start(xT_sb[:, 0, 0:2, :], xT[:, 0, 0:2, :])
            nc.sync.dma_start(wk_sb[:, 0, :, :], wk[:, 0, :, :])
            nc.sync.dma_start(xT_sb[:, 0, 2:5, :], xT[:, 0, 2:5, :])
            nc.scalar.dma_start(bq_sb[:], bq[:])
            nc.scalar.dma_start(bk_sb[:], bk[:])
            nc.scalar.dma_start(bv_sb[:], bv[:].to_broadcast((128, 260)))
            nc.scalar.dma_start(tri_sb[:], tri[:])
            nc.scalar.dma_start(wq_sb[:, 1, :, :], wq[:, 1, :, :])
            nc.scalar.dma_start(wk_sb[:, 1, :, :], wk[:, 1, :, :])
            nc.scalar.dma_start(wv_sb[:], wv[:])
            nc.sync.dma_start(xT_sb[:, 1, :, :], xT[:, 1, :, :])
            nc.scalar.dma_start(wp_sb[:], wp[:])
            nc.sync.dma_start(xT_sb[:, 2, :, :], xT[:, 2, :, :])
            nc.sync.dma_start(xT_sb[:, 3, :, :], xT[:, 3, :, :])

            for h in range(HC):
                lo = 64 * (h % 2)
                nc.gpsimd.memset(kTz[h][64 - lo : 128 - lo, :], 0.0)

            # ---- phase emitters ---------------------------------------
            def emit_proj_qk(tcol, halves=(0, 1)):
                """q/k projections for one 512-token column tile."""
                csl = bass.ds(tcol * 512, 512)
                for half in halves:  # heads (0,1) then (2,3)
                    ps_q = qkps.tile([128, 512], F32, tag="po", name="ps_q", bufs=2)
                    for ks in range(KS):
                        nc.tensor.matmul(
                            ps_q[:],
                            wq_sb[:, half, ks, :],
                            xT_sb[:, tcol, ks, :],
                            start=(ks == 0),
                            stop=(ks == KS - 1),
                        )
                    nc.vector.tensor_scalar_add(
                        qT[half][:, csl], ps_q[:], bq_sb[:, half : half + 1]
                    )
                    ps_k = qkps.tile([128, 512], F32, tag="po", name="ps_k", bufs=2)
                    for ks in range(KS):
                        nc.tensor.matmul(
                            ps_k[:],
                            wk_sb[:, half, ks, :],
                            xT_sb[:, tcol, ks, :],
                            start=(ks == 0),
                            stop=(ks == KS - 1),
                        )
                    h0, h1 = 2 * half, 2 * half + 1
                    nc.vector.tensor_scalar_add(
                        kTz[h0][0:64, csl], ps_k[0:64, :], bk_sb[0:64, half : half + 1]
                    )
                    nc.vector.tensor_scalar_add(
                        kTz[h1][64:128, csl],
                        ps_k[64:128, :],
                        bk_sb[64:128, half : half + 1],
                    )

            def emit_proj_v(tcol):
                """v projections for one 512-token column tile."""
                for jj in range(4):  # v tiles, one 128-token tile each
                    tt = tcol * 4 + jj
                    ps_v = qkps.tile([128, 512], F32, tag="po", name="ps_v", bufs=2)
                    for ks in range(KS):
                        nc.tensor.matmul(
                            ps_v[:, :260],
                            xT_sb[:, tcol, ks, bass.ds(jj * 128, 128)],
                            wv_sb[:, ks, :],
                            start=(ks == 0),
                            stop=(ks == KS - 1),
                        )
                    # bias add also writes the ones columns (65h+64);
                    # DVE not GpSimd: GPSIMD cannot read PSUM
                    nc.vector.tensor_add(v_sb[:, tt, :], ps_v[:, :260], bv_sb[:])

            ex_tiles = {}  # (h, g) -> list of exp pair tiles

            def emit_scores(h, g):
                """Scores + exp + diagonal masks for head h, query group g."""
                nkj = 4 * g + 4
                exl = []
                for j in range(0, nkj, 2):  # kj pairs
                    qk = qkps.tile([128, 2, 512], F32, tag="qk", name=f"qk_h{h}")
                    for i2 in range(2):
                        kj = j + i2
                        ksl = bass.ds(kj * 128, 128)
                        # queries strictly below kj contribute nothing
                        ri = max(0, kj - 4 * g)
                        qsl = bass.ds(g * 512 + ri * 128, 512 - ri * 128)
                        nc.tensor.matmul(
                            qk[:, i2, bass.ds(ri * 128, 512 - ri * 128)],
                            kTz[h][:, ksl],
                            qT[h // 2][:, qsl],
                            start=True,
                            stop=True,
                        )
                    # queries below kj are fully masked; skip them
                    rlo = max(0, j - 4 * g)
                    esl = bass.ds(rlo * 128, 512 - rlo * 128)
                    ex = expp.tile([128, 2, 512], BF16, tag="exp")
                    nc.scalar.activation(
                        ex[:, :, esl], qk[:, :, esl], ACT_F.Exp, scale=0.125
                    )
                    exl.append(ex)
                    # zero the masked triangle on diagonal blocks as soon
                    # as their pair's exp lands (diag qi=4g+r lives in
                    # pair 2g + r//2, i.e. the last two pairs)
                    p = j // 2
                    if p >= 2 * g:
                        for r in (0, 1) if p == 2 * g else (2, 3):
                            qi = 4 * g + r
                            dsl = bass.ds(r * 128, 128)
                            nc.gpsimd.tensor_mul(
                                ex[:, qi % 2, dsl], ex[:, qi % 2, dsl], tri_sb[:]
                            )
                ex_tiles[(h, g)] = exl

            _ctx_cache = {}

            def _ctx_for(g):
                if g not in _ctx_cache:
                    _ctx_cache[g] = ctxp.tile(
                        [128, 4, 256], BF16, tag="ctx", name=f"ctx{g}"
                    )
                return _ctx_cache[g]

            def emit_av(h, g):
                """AV + normalize for head h, group g; fills ctx (+ctxT).

                Even heads batch all 4 chains into one psum bank (single
                reciprocal); odd heads run per-r so each token tile's
                transpose can issue as soon as its pair of norms lands,
                spreading the sync-queue transposes out in time."""
                exl = ex_tiles.pop((h, g))
                ctx_t = _ctx_for(g)
                hsl = bass.ds(64 * h, 64)
                av = qkps.tile([128, 4, 66], F32, tag="av", bufs=2)
                for r in range(4):
                    qi = 4 * g + r
                    for kj in range(qi + 1):
                        nc.tensor.matmul(
                            av[:, r, 0:65],
                            exl[kj // 2][:, kj % 2, bass.ds(r * 128, 128)],
                            v_sb[:, kj, bass.ds(65 * h, 65)],
                            start=(kj == 0),
                            stop=(kj == qi),
                        )
                rd = rdp.tile([128, 4], F32, tag="rd")
                nc.vector.reciprocal(rd[:], av[:, :, 64:65])
                for r in range(4):
                    nc.vector.tensor_scalar_mul(
                        ctx_t[:, r, hsl], av[:, r, 0:64], rd[:, r : r + 1]
                    )
                if h % 2 == 1:  # heads (h-1, h) pair complete -> transpose
                    half = h // 2
                    for r in range(4):
                        tt = 4 * g + r
                        nc.sync.dma_start(
                            ctxT[half][:, tt, :],
                            ctx_t[:, r, bass.ds(128 * half, 128)],
                            transpose=True,
                        )

            def emit_out_pair(g, p):
                """Output projection + 512KB DMA for tiles (2p, 2p+1) of
                group g."""
                og = outp.tile([128, 2, D], BF16, tag="og", name="og")
                for rr in range(2):
                    r = 2 * p + rr
                    tt = g * 4 + r
                    for half in range(2):
                        po = qkps.tile([128, 512], F32, tag="po", name="ps_o", bufs=2)
                        for i in range(2):
                            nc.tensor.matmul(
                                po[:],
                                ctxT[i][:, tt, :],
                                wp_sb[:, i, bass.ds(half * 512, 512)],
                                start=(i == 0),
                                stop=(i == 1),
                            )
                        nc.vector.tensor_copy(
                            og[:, rr, bass.ds(half * 512, 512)], po[:]
                        )
                nc.gpsimd.dma_start(
                    out[:, bass.ds((g * 4 + 2 * p) * D, 2 * D)], og[:]
                )
                if p == 1:
                    _ctx_cache.pop(g, None)

            def emit_drain(g, fillers={}):
                """Fused drain for the LAST group: per-r AV chain for the
                final head -> norm -> transpose -> out-proj -> half DMAs.
                fillers[r] emits extra PE work after tile r's projection
                to cover the tail exps."""
                h = HC - 1
                exl = ex_tiles.pop((h, g))
                ctx_t = _ctx_for(g)
                hsl = bass.ds(64 * h, 64)
                for r in range(4):
                    qi = 4 * g + r
                    av = qkps.tile([128, 4, 66], F32, tag="av", name="av_dr", bufs=2)
                    for kj in range(qi + 1):
                        nc.tensor.matmul(
                            av[:, 0, 0:65],
                            exl[kj // 2][:, kj % 2, bass.ds(r * 128, 128)],
                            v_sb[:, kj, bass.ds(65 * h, 65)],
                            start=(kj == 0),
                            stop=(kj == qi),
                        )
                    rd = rdp.tile([128, 4], F32, tag="rd", name="rd_dr")
                    nc.vector.reciprocal(rd[:, 0:1], av[:, 0, 64:65])
                    nc.vector.tensor_scalar_mul(
                        ctx_t[:, r, hsl], av[:, 0, 0:64], rd[:, 0:1]
                    )
                    tt = 4 * g + r
                    nc.sync.dma_start(
                        ctxT[1][:, tt, :],
                        ctx_t[:, r, bass.ds(128, 128)],
                        transpose=True,
                    )
                    ot = outp.tile([128, D], BF16, tag="ot", name="ot", bufs=2)
                    for half in range(2):
                        po = qkps.tile([128, 512], F32, tag="po", name="ps_o", bufs=2)
                        for i in range(2):
                            nc.tensor.matmul(
                                po[:],
                                ctxT[i][:, tt, :],
                                wp_sb[:, i, bass.ds(half * 512, 512)],
                                start=(i == 0),
                                stop=(i == 1),
                            )
                        osl = bass.ds(half * 512, 512)
                        # split drain casts: ScalarE is free after exps
                        if half == 1:
                            nc.scalar.copy(ot[:, osl], po[:])
                        else:
                            nc.vector.tensor_copy(ot[:, osl], po[:])
                        # rotate queues so the final half-tile transfers
                        # drain in parallel instead of serializing on one
                        eng = (nc.gpsimd, nc.sync)[(2 * r + half) % 2]
                        eng.dma_start(
                            out[:, bass.ds(tt * D + half * 512, 512)], ot[:, osl]
                        )
                    if r in fillers:
                        fillers[r]()
                _ctx_cache.pop(g, None)

            # ---- schedule ---------------------------------------------
            # Rounds by query group; scores/AV software-pipelined across
            # heads; projections and lagged output groups fill the PE
            # while ScalarE streams exps. O(g-1) sits between AV(1,g) and
            # S(3,g) so its DVE casts queue after the norms feeding the
            # g-h01 transposes but before the AV(2/3,g) norms. S(0,3) is
            # pulled into round 2's tail to give ScalarE a head start on
            # the exp-heaviest round.
            # round 0: heads (0,1) attention needs only the half-0
            # projection chains, so the exp stream starts ~5us earlier
            emit_proj_qk(0, halves=(0,))
            emit_scores(0, 0)
            emit_proj_qk(0, halves=(1,))
            emit_scores(1, 0)
            emit_proj_v(0)
            emit_av(0, 0); emit_scores(2, 0); emit_proj_qk(1)
            emit_av(1, 0); emit_scores(3, 0); emit_proj_v(1)
            emit_av(2, 0); emit_av(3, 0)
            # round 1
            emit_scores(0, 1); emit_proj_qk(2); emit_scores(1, 1)
            emit_av(0, 1); emit_scores(2, 1); emit_proj_v(2)
            emit_av(1, 1); emit_scores(3, 1); emit_av(2, 1)
            emit_out_pair(0, 0); emit_av(3, 1); emit_out_pair(0, 1)
            # round 2
            emit_scores(0, 2); emit_proj_qk(3); emit_scores(1, 2)
            emit_av(0, 2); emit_scores(2, 2); emit_proj_v(3)
            emit_av(1, 2); emit_scores(3, 2); emit_av(2, 2)
            emit_out_pair(1, 0); emit_scores(0, 3); emit_av(3, 2)
            emit_out_pair(1, 1)
            # round 3
            emit_scores(1, 3); emit_av(0, 3)
            emit_scores(2, 3); emit_av(1, 3)
            emit_scores(3, 3); emit_out_pair(2, 0); emit_av(2, 3)
            emit_out_pair(2, 1)
            emit_drain(3)

    return nc


_NC_CACHE = None


def _get_nc():
    global _NC_CACHE
    if _NC_CACHE is None:
        nc = _build_nc()
        nc.finalize()  # runs Bacc's pass pipeline (sync-wait splitting etc.)
        _NC_CACHE = nc
    return _NC_CACHE


def _make_in_maps(x, W_qkv, b_qkv, W_proj):
    tri = np.triu(np.ones((128, 128), dtype=np.float32)).astype(NPBF16)

    def wtile(w):  # [D, M] -> [128, KS, M] contraction-major tiles
        m = w.shape[1]
        return np.ascontiguousarray(
            w.astype(NPBF16).reshape(KS, 128, m).transpose(1, 0, 2)
        )

    def wtile2(w):  # [D, 256] -> [128, 2 halves, KS, 128] half-major
        return np.ascontiguousarray(
            w.astype(NPBF16).reshape(KS, 128, 2, 128).transpose(1, 2, 0, 3)
        )

    # xT per batch: [S, D] -> [128, NCOL, KS, 512]
    xTs = [
        np.ascontiguousarray(
            x[b]
            .astype(NPBF16)
            .reshape(NCOL, 512, KS, 128)
            .transpose(3, 0, 2, 1)
        )
        for b in range(B)
    ]

    in_maps = []
    for c in range(8):
        b = c // 4
        hs = [4 * (c % 4) + i for i in range(HC)]
        cs = np.concatenate([np.arange(64 * h, 64 * h + 64) for h in hs])
        wq_c = W_qkv[:, 0 * D :][:, cs]                      # [D, 256]
        wk_c = W_qkv[:, 1 * D :][:, cs]
        v_blk = W_qkv[:, 2 * D :][:, cs].astype(np.float32)  # [D, 256]
        wv_c = np.zeros((D, 260), dtype=np.float32)
        bv_c = np.zeros((1, 260), dtype=np.float32)
        for i in range(HC):
            wv_c[:, 65 * i : 65 * i + 64] = v_blk[:, 64 * i : 64 * i + 64]
            bv_c[0, 65 * i : 65 * i + 64] = b_qkv[2 * D :][cs][64 * i : 64 * i + 64]
            bv_c[0, 65 * i + 64] = 1.0
        in_maps.append(
            {
                "xT": xTs[b],
                "wq": wtile2(wq_c),
                "wk": wtile2(wk_c),
                "wv": wtile(wv_c),
                "bq": np.ascontiguousarray(
                    b_qkv[0 * D :][cs].astype(np.float32).reshape(2, 128).T
                ),
                "bk": np.ascontiguousarray(
                    b_qkv[1 * D :][cs].astype(np.float32).reshape(2, 128).T
                ),
                "bv": bv_c.astype(NPBF16),
                "wp": np.ascontiguousarray(
                    W_proj[cs, :].astype(NPBF16).reshape(2, 128, D).transpose(1, 0, 2)
                ),
                "tri": tri,
            }
        )
    return in_maps


def kernel(x, W_qkv, b_qkv, W_proj, b_proj, **run_kwargs):
    x = np.asarray(x, dtype=np.float32)
    W_qkv = np.asarray(W_qkv, dtype=np.float32)
    b_qkv = np.asarray(b_qkv, dtype=np.float32)
    W_proj = np.asarray(W_proj, dtype=np.float32)
    b_proj = np.asarray(b_proj, dtype=np.float32)

    nc = _get_nc()
    in_maps = _make_in_maps(x, W_qkv, b_qkv, W_proj)
    res = run_bass_kernel_spmd(nc, in_maps, core_ids=list(range(8)), **run_kwargs)

    out = np.zeros((B, S, D), dtype=np.float32)
    for c in range(8):
        b = c // 4
        # o[p, tt*D + d] = out[tt*128 + p, d]
        o = res.results[c]["o"].astype(np.float32).reshape(128, QT, D)
        out[b] += o.transpose(1, 0, 2).reshape(S, D)
    out += b_proj[None, None, :]
    kernel.last_result = res
    return out


# revision 32
# speedup vs baseline: 1.1366x; 1.1366x over previous
"""Multi-head causal attention (B=2, S=2048, D=1024, H=16) on 8 NeuronCores.

Sharding v3: 2-way data parallel over batch x 4-way tensor parallel over
heads (core c handles batch c//4, heads 4*(c%4)..4*(c%4)+3). Each core
computes q/k/v projections for its 4 heads over its batch's 2048 tokens,
causal attention, and a partial output projection (its 256 rows of
W_proj); the host sums 4 partials per batch and adds b_proj.

Device-side design (all matmuls bf16 with fp32 PSUM accumulate):
 - x arrives pre-transposed and tiled [128, 4 col-tiles, 8 ks, 512] so
   every input DMA chunk is >=2KB-contiguous per partition.
 - q is produced transposed in two 2-head tiles ([128, 2048] each); k
   likewise but stored as FOUR zero-padded copies (head h in rows
   64*(h%2).., zeros elsewhere) so each head's score matmul contracts
   K=128 at the full-array rate.
 - scores are computed as ST = K @ Q^T ([keys, queries], 512-wide query
   groups); one Exp instruction per kj-pair psum tile. exp runs on
   ScalarE; q/k/v copyback on DVE; causal-triangle masking on the
   otherwise-idle GpSimd, emitted per-pair so AV never waits on it.
 - v is produced token-major [tokens, 260] = [V_h|1]x4; the AV product
   expST.T @ [V|1] yields context AND the softmax denominator in one
   accumulation group with queries on PSUM partitions. The 4 query
   tiles of a group accumulate into ONE psum bank ([128,4,66]) so a
   single reciprocal serves all 4 normalizations.
 - scores/AV are software-pipelined ACROSS HEADS; lagged output groups
   are emitted between AV(1,g) and S(3,g) so their DVE casts queue
   after the norms that feed the next transposes (avoids a DVE-order
   priority inversion that stalls the output projection).
 - output is stored partition-major ([128, 16*1024] per core) so out
   DMAs have 2-8KB contiguous descriptors; the host undoes the tiling.
   The last group runs a fused per-tile drain: AV chain -> norm ->
   transpose -> out-proj -> half-tile DMA, pipelined per token tile.
"""

import sys

sys.path.insert(0, "/opt/trn_rl_repo")

import numpy as np
import ml_dtypes

import concourse.bass as bass
import concourse.mybir as mybir
import concourse.tile as tile
from concourse import bacc
from concourse.bass_utils import run_bass_kernel_spmd

BF16 = mybir.dt.bfloat16
F32 = mybir.dt.float32
NPBF16 = ml_dtypes.bfloat16

B, S, D = 2, 2048, 1024
H, DH = 16, 64
HC = 4               # heads per core
T = S                # tokens per core (one batch)
KS = D // 128        # 8 contraction subtiles
QT = T // 128        # 16 query tiles
NCOL = 4             # 512-token projection column tiles
ACT_F = mybir.ActivationFunctionType


def _build_nc():
    # Bacc (not raw Bass): its compile() pass pipeline splits multi-sem
    # waits down to the TRN2 1-wait-per-instruction hardware limit.
    nc = bacc.Bacc("TRN2", target_bir_lowering=False, debug=False, num_devices=8)

    xT = nc.dram_tensor("xT", [128, NCOL, KS, 512], BF16, kind="ExternalInput")
    wq = nc.dram_tensor("wq", [128, 2, KS, 128], BF16, kind="ExternalInput")
    wk = nc.dram_tensor("wk", [128, 2, KS, 128], BF16, kind="ExternalInput")
    wv = nc.dram_tensor("wv", [128, KS, 260], BF16, kind="ExternalInput")
    bq = nc.dram_tensor("bq", [128, 2], F32, kind="ExternalInput")
    bk = nc.dram_tensor("bk", [128, 2], F32, kind="ExternalInput")
    bv = nc.dram_tensor("bv", [1, 260], BF16, kind="ExternalInput")
    wp = nc.dram_tensor("wp", [128, 2, D], BF16, kind="ExternalInput")
    tri = nc.dram_tensor("tri", [128, 128], BF16, kind="ExternalInput")
    # partition-major output: o[p, tt*1024 + d] = out[tt*128 + p, d]
    out = nc.dram_tensor("o", [128, QT * D], BF16, kind="ExternalOutput")

    with tile.TileContext(nc) as tc:
        with (
            tc.tile_pool(name="singles", bufs=1) as singles,
            # one psum pool: tag "qk" [128,2,512] f32 = 2 banks x 2 bufs,
            # tag "av" [128,4,66] = 1 bank x 2, tag "po" [128,512] = 1 bank
            # x 2 -> exactly 8 banks
            tc.tile_pool(name="qkps", bufs=2, space="PSUM") as qkps,
            tc.tile_pool(name="expp", bufs=36) as expp,
            tc.tile_pool(name="ctxp", bufs=6) as ctxp,
            tc.tile_pool(name="outp", bufs=2) as outp,
            tc.tile_pool(name="rdp", bufs=4) as rdp,
        ):
            # ---- resident tensors -------------------------------------
            wq_sb = singles.tile([128, 2, KS, 128], BF16, tag="wq")
            wk_sb = singles.tile([128, 2, KS, 128], BF16, tag="wk")
            wv_sb = singles.tile([128, KS, 260], BF16, tag="wv")
            bq_sb = singles.tile([128, 2], F32, tag="bq")
            bk_sb = singles.tile([128, 2], F32, tag="bk")
            # b_v (+ the ones columns) broadcast to all partitions; fused
            # into the v copyback as a tensor_tensor add on DVE
            bv_sb = singles.tile([128, 260], BF16, tag="bv")
            wp_sb = singles.tile([128, 2, D], BF16, tag="wp")
            tri_sb = singles.tile([128, 128], BF16, tag="tri")
            xT_sb = singles.tile([128, NCOL, KS, 512], BF16, tag="xT")
            # q for heads (0,1) in qT[0] rows (0:64|64:128), (2,3) in qT[1]
            qT = [
                singles.tile([128, T], BF16, tag=f"qT{i}", name=f"qT{i}")
                for i in range(2)
            ]
            # kT stored 4x, zero-padded per head (see module docstring)
            kTz = [
                singles.tile([128, T], BF16, tag=f"kTz{h}", name=f"kTz{h}")
                for h in range(HC)
            ]
            # v, per key-tile: [V_h0 | 1 | V_h1 | 1 | V_h2 | 1 | V_h3 | 1]
            v_sb = singles.tile([128, QT, 260], BF16, tag="v")
            # ctxT: dims of heads (0,1) in [0], (2,3) in [1]; matches wp rows
            ctxT = [
                singles.tile([128, QT, 128], BF16, tag=f"ctxT{i}", name=f"ctxT{i}")
                for i in range(2)
            ]

            # ---- input DMA: sync queue carries everything the first
            # q-projection chain needs, in need-order (wq then xT col 0
            # then later cols); scalar queue carries the small consts
            # then the big weights needed a bit later (wk by ~15us)
            # three queues pull the critical first-column data in
            # parallel: sync carries wqA/wkA + the front of xT col 0,
            # gpsimd (25ns issue, before its memsets) the back half of
            # col 0, scalar the consts then half-1 weights then wv
            nc.gpsimd.dma_start(xT_sb[:, 0, 5:, :], xT[:, 0, 5:, :])
            nc.sync.dma_start(wq_sb[:, 0, :, :], wq[:, 0, :, :])
            nc.sync.dma_start(xT_sb[:, 0, 0:2, :], xT[:, 0, 0:2, :])
            nc.sync.dma_start(wk_sb[:, 0, :, :], wk[:, 0, :, :])
            nc.sync.dma_start(xT_sb[:, 0, 2:5, :], xT[:, 0, 2:5, :])
            nc.scalar.dma_start(bq_sb[:], bq[:])
            nc.scalar.dma_start(bk_sb[:], bk[:])
            nc.scalar.dma_start(bv_sb[:], bv[:].to_broadcast((128, 260)))
            nc.scalar.dma_start(tri_sb[:], tri[:])
            nc.scalar.dma_start(wq_sb[:, 1, :, :], wq[:, 1, :, :])
            nc.scalar.dma_start(wk_sb[:, 1, :, :], wk[:, 1, :, :])
            nc.scalar.dma_start(wv_sb[:], wv[:])
            nc.sync.dma_start(xT_sb[:, 1, :, :], xT[:, 1, :, :])
            nc.scalar.dma_start(wp_sb[:], wp[:])
            nc.sync.dma_start(xT_sb[:, 2, :, :], xT[:, 2, :, :])
            nc.sync.dma_start(xT_sb[:, 3, :, :], xT[:, 3, :, :])

            for h in range(HC):
                lo = 64 * (h % 2)
                nc.gpsimd.memset(kTz[h][64 - lo : 128 - lo, :], 0.0)

            # ---- phase emitters ---------------------------------------
            def emit_proj_qk(tcol, halves=(0, 1)):
                """q/k projections for one 512-token column tile."""
                csl = bass.ds(tcol * 512, 512)
                for half in halves:  # heads (0,1) then (2,3)
                    ps_q = qkps.tile([128, 512], F32, tag="po", name="ps_q", bufs=2)
                    for ks in range(KS):
                        nc.tensor.matmul(
                            ps_q[:],
                            wq_sb[:, half, ks, :],
                            xT_sb[:, tcol, ks, :],
                            start=(ks == 0),
                            stop=(ks == KS - 1),
                        )
                    nc.vector.tensor_scalar_add(
                        qT[half][:, csl], ps_q[:], bq_sb[:, half : half + 1]
                    )
                    ps_k = qkps.tile([128, 512], F32, tag="po", name="ps_k", bufs=2)
                    for ks in range(KS):
                        nc.tensor.matmul(
                            ps_k[:],
                            wk_sb[:, half, ks, :],
                            xT_sb[:, tcol, ks, :],
                            start=(ks == 0),
                            stop=(ks == KS - 1),
                        )
                    h0, h1 = 2 * half, 2 * half + 1
                    nc.vector.tensor_scalar_add(
                        kTz[h0][0:64, csl], ps_k[0:64, :], bk_sb[0:64, half : half + 1]
                    )
                    nc.vector.tensor_scalar_add(
                        kTz[h1][64:128, csl],
                        ps_k[64:128, :],
                        bk_sb[64:128, half : half + 1],
                    )

            def emit_proj_v(tcol):
                """v projections for one 512-token column tile."""
                for jj in range(4):  # v tiles, one 128-token tile each
                    tt = tcol * 4 + jj
                    ps_v = qkps.tile([128, 512], F32, tag="po", name="ps_v", bufs=2)
                    for ks in range(KS):
                        nc.tensor.matmul(
                            ps_v[:, :260],
                            xT_sb[:, tcol, ks, bass.ds(jj * 128, 128)],
                            wv_sb[:, ks, :],
                            start=(ks == 0),
                            stop=(ks == KS - 1),
                        )
                    # bias add also writes the ones columns (65h+64);
                    # DVE not GpSimd: GPSIMD cannot read PSUM
                    nc.vector.tensor_add(v_sb[:, tt, :], ps_v[:, :260], bv_sb[:])

            ex_tiles = {}  # (h, g) -> list of exp pair tiles

            def emit_scores(h, g):
                """Scores + exp + diagonal masks for head h, query group g."""
                nkj = 4 * g + 4
                exl = []
                for j in range(0, nkj, 2):  # kj pairs
                    qk = qkps.tile([128, 2, 512], F32, tag="qk", name=f"qk_h{h}")
                    for i2 in range(2):
                        kj = j + i2
                        ksl = bass.ds(kj * 128, 128)
                        # queries strictly below kj contribute nothing
                        ri = max(0, kj - 4 * g)
                        qsl = bass.ds(g * 512 + ri * 128, 512 - ri * 128)
                        nc.tensor.matmul(
                            qk[:, i2, bass.ds(ri * 128, 512 - ri * 128)],
                            kTz[h][:, ksl],
                            qT[h // 2][:, qsl],
                            start=True,
                            stop=True,
                        )
                    # queries below kj are fully masked; skip them
                    rlo = max(0, j - 4 * g)
                    esl = bass.ds(rlo * 128, 512 - rlo * 128)
                    ex = expp.tile([128, 2, 512], BF16, tag="exp")
                    nc.scalar.activation(
                        ex[:, :, esl], qk[:, :, esl], ACT_F.Exp, scale=0.125
                    )
                    exl.append(ex)
                    # zero the masked triangle on diagonal blocks as soon
                    # as their pair's exp lands (diag qi=4g+r lives in
                    # pair 2g + r//2, i.e. the last two pairs)
                    p = j // 2
                    if p >= 2 * g:
                        for r in (0, 1) if p == 2 * g else (2, 3):
                            qi = 4 * g + r
                            dsl = bass.ds(r * 128, 128)
                            nc.gpsimd.tensor_mul(
                                ex[:, qi % 2, dsl], ex[:, qi % 2, dsl], tri_sb[:]
                            )
                ex_tiles[(h, g)] = exl

            _ctx_cache = {}

            def _ctx_for(g):
                if g not in _ctx_cache:
                    _ctx_cache[g] = ctxp.tile(
                        [128, 4, 256], BF16, tag="ctx", name=f"ctx{g}"
                    )
                return _ctx_cache[g]

            def emit_av(h, g):
                """AV + normalize for head h, group g; fills ctx (+ctxT).

                Even heads batch all 4 chains into one psum bank (single
                reciprocal); odd heads run per-r so each token tile's
                transpose can issue as soon as its pair of norms lands,
                spreading the sync-queue transposes out in time."""
                exl = ex_tiles.pop((h, g))
                ctx_t = _ctx_for(g)
                hsl = bass.ds(64 * h, 64)
                av = qkps.tile([128, 4, 66], F32, tag="av", bufs=2)
                for r in range(4):
                    qi = 4 * g + r
                    for kj in range(qi + 1):
                        nc.tensor.matmul(
                            av[:, r, 0:65],
                            exl[kj // 2][:, kj % 2, bass.ds(r * 128, 128)],
                            v_sb[:, kj, bass.ds(65 * h, 65)],
                            start=(kj == 0),
                            stop=(kj == qi),
                        )
                rd = rdp.tile([128, 4], F32, tag="rd")
                nc.vector.reciprocal(rd[:], av[:, :, 64:65])
                for r in range(4):
                    nc.vector.tensor_scalar_mul(
                        ctx_t[:, r, hsl], av[:, r, 0:64], rd[:, r : r + 1]
                    )
                if h % 2 == 1:  # heads (h-1, h) pair complete -> transpose
                    half = h // 2
                    for r in range(4):
                        tt = 4 * g + r
                        nc.sync.dma_start(
                            ctxT[half][:, tt, :],
                            ctx_t[:, r, bass.ds(128 * half, 128)],
                            transpose=True,
                        )

            def emit_out_pair(g, p):
                """Output projection + 512KB DMA for tiles (2p, 2p+1) of
                group g."""
                og = outp.tile([128, 2, D], BF16, tag="og", name="og")
                for rr in range(2):
                    r = 2 * p + rr
                    tt = g * 4 + r
                    for half in range(2):
                        po = qkps.tile([128, 512], F32, tag="po", name="ps_o", bufs=2)
                        for i in range(2):
                            nc.tensor.matmul(
                                po[:],
                                ctxT[i][:, tt, :],
                                wp_sb[:, i, bass.ds(half * 512, 512)],
                                start=(i == 0),
                                stop=(i == 1),
                            )
                        nc.vector.tensor_copy(
                            og[:, rr, bass.ds(half * 512, 512)], po[:]
                        )
                nc.gpsimd.dma_start(
                    out[:, bass.ds((g * 4 + 2 * p) * D, 2 * D)], og[:]
                )
                if p == 1:
                    _ctx_cache.pop(g, None)

            def emit_drain(g, fillers={}):
                """Fused drain for the LAST group: per-r AV chain for the
                final head -> norm -> transpose -> out-proj -> half DMAs.
                fillers[r] emits extra PE work after tile r's projection
                to cover the tail exps."""
                h = HC - 1
                exl = ex_tiles.pop((h, g))
                ctx_t = _ctx_for(g)
                hsl = bass.ds(64 * h, 64)
                for r in range(4):
                    qi = 4 * g + r
                    av = qkps.tile([128, 4, 66], F32, tag="av", name="av_dr", bufs=2)
                    for kj in range(qi + 1):
                        nc.tensor.matmul(
                            av[:, 0, 0:65],
                            exl[kj // 2][:, kj % 2, bass.ds(r * 128, 128)],
                            v_sb[:, kj, bass.ds(65 * h, 65)],
                            start=(kj == 0),
                            stop=(kj == qi),
                        )
                    rd = rdp.tile([128, 4], F32, tag="rd", name="rd_dr")
                    nc.vector.reciprocal(rd[:, 0:1], av[:, 0, 64:65])
                    nc.vector.tensor_scalar_mul(
                        ctx_t[:, r, hsl], av[:, 0, 0:64], rd[:, 0:1]
                    )
                    tt = 4 * g + r
                    nc.sync.dma_start(
                        ctxT[1][:, tt, :],
                        ctx_t[:, r, bass.ds(128, 128)],
                        transpose=True,
                    )
                    ot = outp.tile([128, D], BF16, tag="ot", name="ot", bufs=2)
                    for half in range(2):
                        po = qkps.tile([128, 512], F32, tag="po", name="ps_o", bufs=2)
                        for i in range(2):
                            nc.tensor.matmul(
                                po[:],
                                ctxT[i][:, tt, :],
                                wp_sb[:, i, bass.ds(half * 512, 512)],
                                start=(i == 0),
                                stop=(i == 1),
                            )
                        osl = bass.ds(half * 512, 512)
                        # split drain casts: ScalarE is free after exps
                        if half == 1:
                            nc.scalar.copy(ot[:, osl], po[:])
                        else:
                            nc.vector.tensor_copy(ot[:, osl], po[:])
                        # rotate queues so the final half-tile transfers
                        # drain in parallel instead of serializing on one
                        eng = (nc.gpsimd, nc.sync)[(2 * r + half) % 2]
                        eng.dma_start(
                            out[:, bass.ds(tt * D + half * 512, 512)], ot[:, osl]
                        )
                    if r in fillers:
                        fillers[r]()
                _ctx_cache.pop(g, None)

            # ---- schedule ---------------------------------------------
            # Rounds by query group; scores/AV software-pipelined across
            # heads; projections and lagged output groups fill the PE
            # while ScalarE streams exps. O(g-1) sits between AV(1,g) and
            # S(3,g) so its DVE casts queue after the norms feeding the
            # g-h01 transposes but before the AV(2/3,g) norms. S(0,3) is
            # pulled into round 2's tail to give ScalarE a head start on
            # the exp-heaviest round.
            # round 0: heads (0,1) attention needs only the half-0
            # projection chains, so the exp stream starts ~5us earlier
            emit_proj_qk(0, halves=(0,))
            emit_scores(0, 0)
            emit_proj_qk(0, halves=(1,))
            emit_scores(1, 0)
            emit_proj_v(0)
            emit_av(0, 0); emit_scores(2, 0); emit_proj_qk(1)
            emit_av(1, 0); emit_scores(3, 0); emit_proj_v(1)
            emit_av(2, 0); emit_av(3, 0)
            # round 1
            emit_scores(0, 1); emit_proj_qk(2); emit_scores(1, 1)
            emit_av(0, 1); emit_scores(2, 1); emit_proj_v(2)
            emit_av(1, 1); emit_scores(3, 1); emit_av(2, 1)
            emit_out_pair(0, 0); emit_av(3, 1); emit_out_pair(0, 1)
            # round 2
            emit_scores(0, 2); emit_proj_qk(3); emit_scores(1, 2)
            emit_av(0, 2); emit_scores(2, 2); emit_proj_v(3)
            emit_av(1, 2); emit_scores(3, 2); emit_av(2, 2)
            emit_out_pair(1, 0); emit_scores(0, 3); emit_av(3, 2)
            emit_out_pair(1, 1)
            # round 3
            emit_scores(1, 3); emit_av(0, 3)
            emit_scores(2, 3); emit_av(1, 3)
            emit_out_pair(2, 0); emit_scores(3, 3); emit_av(2, 3)
            emit_out_pair(2, 1)
            emit_drain(3)

    return nc


_NC_CACHE = None


def _get_nc():
    global _NC_CACHE
    if _NC_CACHE is None:
        nc = _build_nc()
        nc.finalize()  # runs Bacc's pass pipeline (sync-wait splitting etc.)
        _NC_CACHE = nc
    return _NC_CACHE


def _make_in_maps(x, W_qkv, b_qkv, W_proj):
    tri = np.triu(np.ones((128, 128), dtype=np.float32)).astype(NPBF16)

    def wtile(w):  # [D, M] -> [128, KS, M] contraction-major tiles
        m = w.shape[1]
        return np.ascontiguousarray(
            w.astype(NPBF16).reshape(KS, 128, m).transpose(1, 0, 2)
        )

    def wtile2(w):  # [D, 256] -> [128, 2 halves, KS, 128] half-major
        return np.ascontiguousarray(
            w.astype(NPBF16).reshape(KS, 128, 2, 128).transpose(1, 2, 0, 3)
        )

    # xT per batch: [S, D] -> [128, NCOL, KS, 512]
    xTs = [
        np.ascontiguousarray(
            x[b]
            .astype(NPBF16)
            .reshape(NCOL, 512, KS, 128)
            .transpose(3, 0, 2, 1)
        )
        for b in range(B)
    ]

    in_maps = []
    for c in range(8):
        b = c // 4
        hs = [4 * (c % 4) + i for i in range(HC)]
        cs = np.concatenate([np.arange(64 * h, 64 * h + 64) for h in hs])
        wq_c = W_qkv[:, 0 * D :][:, cs]                      # [D, 256]
        wk_c = W_qkv[:, 1 * D :][:, cs]
        v_blk = W_qkv[:, 2 * D :][:, cs].astype(np.float32)  # [D, 256]
        wv_c = np.zeros((D, 260), dtype=np.float32)
        bv_c = np.zeros((1, 260), dtype=np.float32)
        for i in range(HC):
            wv_c[:, 65 * i : 65 * i + 64] = v_blk[:, 64 * i : 64 * i + 64]
            bv_c[0, 65 * i : 65 * i + 64] = b_qkv[2 * D :][cs][64 * i : 64 * i + 64]
            bv_c[0, 65 * i + 64] = 1.0
        in_maps.append(
            {
                "xT": xTs[b],
                "wq": wtile2(wq_c),
                "wk": wtile2(wk_c),
                "wv": wtile(wv_c),
                "bq": np.ascontiguousarray(
                    b_qkv[0 * D :][cs].astype(np.float32).reshape(2, 128).T
                ),
                "bk": np.ascontiguousarray(
                    b_qkv[1 * D :][cs].astype(np.float32).reshape(2, 128).T
                ),
                "bv": bv_c.astype(NPBF16),
                "wp": np.ascontiguousarray(
                    W_proj[cs, :].astype(NPBF16).reshape(2, 128, D).transpose(1, 0, 2)
                ),
                "tri": tri,
            }
        )
    return in_maps


def kernel(x, W_qkv, b_qkv, W_proj, b_proj, **run_kwargs):
    x = np.asarray(x, dtype=np.float32)
    W_qkv = np.asarray(W_qkv, dtype=np.float32)
    b_qkv = np.asarray(b_qkv, dtype=np.float32)
    W_proj = np.asarray(W_proj, dtype=np.float32)
    b_proj = np.asarray(b_proj, dtype=np.float32)

    nc = _get_nc()
    in_maps = _make_in_maps(x, W_qkv, b_qkv, W_proj)
    res = run_bass_kernel_spmd(nc, in_maps, core_ids=list(range(8)), **run_kwargs)

    out = np.zeros((B, S, D), dtype=np.float32)
    for c in range(8):
        b = c // 4
        # o[p, tt*D + d] = out[tt*128 + p, d]
        o = res.results[c]["o"].astype(np.float32).reshape(128, QT, D)
        out[b] += o.transpose(1, 0, 2).reshape(S, D)
    out += b_proj[None, None, :]
    kernel.last_result = res
    return out
